# revision 8
# baseline (speedup 1.0000x reference)
"""MoE grouped-experts kernel for Trainium2 (8 NeuronCores, expert-parallel).

v3 strategy
-----------
Expert-parallel, 32 experts on 8 cores x 4 slots, with:

- token-exact slot capacities: slot j's capacity = max over cores of the
  assigned expert's token count (no 128-row block quantization; padding
  drops from ~19% to ~2%). Slots sorted by capacity ascending so the
  first slot's xT load (the only non-overlapped DMA) is the smallest.
- fp16 matmul operands everywhere (same 1 col/cycle PE rate as fp32r,
  but half the HBM traffic and 2x faster FWL weight loads).
- GEMM1 (unchanged orientation): hT[m, t] = sum_k gup[k, m] * xT[k, t];
  stationary = gup tile, moving = token columns.
- activation: fg = Silu(alpha * min(gate, 7)) = alpha*glu;
  at = (clip(up) + 1) * fg  -> alpha * a_ref (probs NOT applied here).
- GEMM2 (transposed vs v1): yT[d, t] = sum_k down[k, d] * aT[k, t];
  stationary = down tile, moving = token columns. Output y^T [DIM, SC]
  keeps everything token-column-exact.
- routing probs (ws / alpha) applied on HOST during the combine
  (linear in y), removing the on-device per-token scale entirely.
"""

import math
from contextlib import ExitStack

import numpy as np

N_TOKENS, DIM = 4096, 2048
N_EXPERTS, TOPK, INTER = 32, 4, 1408
ALPHA, LIMIT, LIN_OFFSET = 1.702, 7.0, 1.0

NCORE = 8
NSLOT = N_EXPERTS // NCORE        # 4
KD = DIM // 128                   # 16 contraction tiles for GEMM1
KI = INTER // 128                 # 11 contraction tiles for GEMM2
NDT = DIM // 128                  # 16 output-dim tiles for GEMM2
SLABK = 8                         # k-tiles per xT slab (8.7KB DMA lines)
NSLAB = KD // SLABK               # 2
C_REF = 2 * ((N_TOKENS * TOPK + N_EXPERTS - 1) // N_EXPERTS)  # 1024

_PROG_CACHE: dict = {}


def _token_groups(cap: int):
    """Split cap into balanced moving-dim groups of <= 512 columns."""
    ng = max(1, math.ceil(cap / 512))
    base, rem = divmod(cap, ng)
    sizes = [base + (1 if i < rem else 0) for i in range(ng)]
    out, off = [], 0
    for s in sizes:
        out.append((off, s))
        off += s
    return out


def _build_program(caps: tuple):
    import concourse.bacc as bacc
    import concourse.mybir as mybir
    import concourse.tile as tile
    from concourse.alu_op_type import AluOpType

    F32 = mybir.dt.float32
    F16 = mybir.dt.float16
    capmax = max(caps)
    SC = sum(caps)
    soff = np.concatenate([[0], np.cumsum(caps)]).tolist()
    xt_off = np.concatenate(
        [[0], np.cumsum([128 * KD * c for c in caps])]
    ).tolist()

    nc = bacc.Bacc(None, target_bir_lowering=False, debug=False)
    with ExitStack() as ctx:
        tc = ctx.enter_context(tile.TileContext(nc))
        dram = ctx.enter_context(tc.tile_pool(name="dram", bufs=1, space="DRAM"))
        xt_d = dram.tile([xt_off[-1]], F16, kind="ExternalInput")
        gup_d = dram.tile([NSLOT, 2, KI, 128, KD * 128], F16, kind="ExternalInput")
        down_d = dram.tile([NSLOT, NDT, 128, KI * 128], F16, kind="ExternalInput")
        y_d = dram.tile([NDT, 128, SC], F16, kind="ExternalOutput")
        names = {
            "xt": xt_d.name, "gup": gup_d.name, "down": down_d.name, "y": y_d.name,
        }

        xt_pool = ctx.enter_context(tc.tile_pool(name="xt", bufs=2 * NSLAB))
        gup_pool = ctx.enter_context(tc.tile_pool(name="gup", bufs=8))
        down_pool = ctx.enter_context(tc.tile_pool(name="down", bufs=20))
        at_pool = ctx.enter_context(tc.tile_pool(name="at", bufs=2))
        fg_pool = ctx.enter_context(tc.tile_pool(name="fg", bufs=4))
        tmp_pool = ctx.enter_context(tc.tile_pool(name="tmp", bufs=4))
        yt_pool = ctx.enter_context(tc.tile_pool(name="yt", bufs=4))
        psg1 = ctx.enter_context(tc.tile_pool(name="psg1", bufs=4, space="PSUM"))
        psg2 = ctx.enter_context(tc.tile_pool(name="psg2", bufs=3, space="PSUM"))
        wu_pool = ctx.enter_context(tc.tile_pool(name="wu", bufs=1))
        wu_ps_pool = ctx.enter_context(tc.tile_pool(name="wups", bufs=1, space="PSUM"))

        # PE warmup: ~96 dependency-free matmuls fill the initial xT/gup DMA
        # wait and flip the HAM clock gate to 8/8 before the real stream starts
        wu = wu_pool.tile([128, 16], F16)
        nc.vector.memset(wu[:], 0.0)
        wu_ps = wu_ps_pool.tile([128, 16], F32)
        for _ in range(96):
            nc.tensor.matmul(wu_ps[:16, :], lhsT=wu[:, :16], rhs=wu[:],
                             start=True, stop=True)

        def emit_gemm1(j):
            cap = caps[j]
            groups = _token_groups(cap)

            xts, xt_dmas = [], []
            for s in range(NSLAB):
                t = xt_pool.tile([128, SLABK * capmax], F16, tag="xt")
                base = xt_off[j] + s * 128 * SLABK * cap

                def dma(t=t, base=base, cap=cap):
                    nc.sync.dma_start(
                        out=t[:, :SLABK * cap],
                        in_=xt_d[base: base + 128 * SLABK * cap].rearrange(
                            "(p c) -> p c", p=128
                        ),
                    )
                xt_dmas.append(dma)
                xts.append(t)
            # first slot: interleave the first gup load between the xT slabs
            # so the PE's first matmul isn't queued behind the whole xT image
            if j > 0:
                for d in xt_dmas:
                    d()
                xt_dmas = []
            else:
                xt_dmas[0]()
                xt_dmas = xt_dmas[1:]

            def xt_ap(k, g0, gw, cap=cap, xts=xts):
                t = xts[k // SLABK]
                kk = k % SLABK
                return t[:, kk * cap + g0: kk * cap + g0 + gw]

            at_sb = at_pool.tile([128, KI * capmax], F16, tag="at")

            for i in range(KI):
                pss = []
                for half in (0, 1):  # 0 = gate, 1 = up
                    gsb = gup_pool.tile([128, KD * 128], F16, tag="gup")
                    nc.sync.dma_start(out=gsb[:], in_=gup_d[j, half, i])
                    for d in xt_dmas:  # remaining first-slot slabs
                        d()
                    xt_dmas = []
                    ps_h = [
                        psg1.tile([128, 512], F32, tag="ps1",
                                  name=f"ps1_{j}_{i}_{half}_{gi}")
                        for gi in range(len(groups))
                    ]
                    for k in range(KD):
                        for gi, (g0, gw) in enumerate(groups):
                            nc.tensor.matmul(
                                ps_h[gi][:, :gw],
                                lhsT=gsb[:, k * 128:(k + 1) * 128],
                                rhs=xt_ap(k, g0, gw),
                                start=(k == 0), stop=(k == KD - 1),
                            )
                    pss.append(ps_h)
                for gi, (g0, gw) in enumerate(groups):
                    t0 = tmp_pool.tile([128, 512], F32, tag="t0")
                    nc.vector.tensor_scalar_min(t0[:, :gw], pss[0][gi][:, :gw], LIMIT)
                    fg = fg_pool.tile([128, 512], F32, tag="fg")
                    nc.scalar.activation(
                        fg[:, :gw], t0[:, :gw],
                        mybir.ActivationFunctionType.Silu, scale=ALPHA,
                    )
                    uc = tmp_pool.tile([128, 512], F32, tag="uc")
                    nc.vector.tensor_scalar(
                        uc[:, :gw], pss[1][gi][:, :gw], LIMIT, -LIMIT,
                        AluOpType.min, AluOpType.max,
                    )
                    # at = (clip(up)+1) * alpha*glu   (probs applied on host)
                    nc.vector.scalar_tensor_tensor(
                        at_sb[:, i * cap + g0: i * cap + g0 + gw],
                        uc[:, :gw], LIN_OFFSET, fg[:, :gw],
                        AluOpType.add, AluOpType.mult,
                    )
            return at_sb

        def emit_gemm2(j, at_sb):
            cap = caps[j]
            groups = _token_groups(cap)
            for dt in range(NDT):
                dsb = down_pool.tile([128, KI * 128], F16, tag="down")
                nc.sync.dma_start(out=dsb[:], in_=down_d[j, dt])
                for gi, (g0, gw) in enumerate(groups):
                    ps2 = psg2.tile([128, 512], F32, tag="ps2",
                                    name=f"ps2_{j}_{dt}_{gi}")
                    for k in range(KI):
                        nc.tensor.matmul(
                            ps2[:, :gw],
                            lhsT=dsb[:, k * 128:(k + 1) * 128],
                            rhs=at_sb[:, k * cap + g0: k * cap + g0 + gw],
                            start=(k == 0), stop=(k == KI - 1),
                        )
                    yt = yt_pool.tile([128, 512], F16, tag="yt")
                    nc.scalar.activation(
                        yt[:, :gw], ps2[:, :gw],
                        mybir.ActivationFunctionType.Copy,
                    )
                    nc.sync.dma_start(
                        out=y_d[dt, :, soff[j] + g0: soff[j] + g0 + gw],
                        in_=yt[:, :gw],
                    )

        # software pipeline: GEMM2 of slot j-1 is emitted after GEMM1 of
        # slot j, hiding the at->GEMM2 dependency tail and the slot
        # transition under the next slot's matmul stream
        prev_at = None
        for j in range(NSLOT + 1):
            cur_at = emit_gemm1(j) if j < NSLOT else None
            if j > 0:
                emit_gemm2(j - 1, prev_at)
            prev_at = cur_at
    nc.compile()
    return nc, names


def _route(indices, token_mask, weights):
    """Replicate the reference's permute/capacity semantics on host."""
    idx = np.asarray(indices).astype(np.int64)
    mask = np.asarray(token_mask).astype(bool)
    w = np.asarray(weights).astype(np.float32)
    flat_e = np.where(mask[:, None], idx, -1).ravel()
    w_flat = np.where(flat_e >= 0, w.ravel(), 0.0).astype(np.float32)

    per_expert = []  # (flat_ids, token_ids, weights), flat order, capped at C_REF
    tok = np.repeat(np.arange(N_TOKENS, dtype=np.int64), TOPK)
    for e in range(N_EXPERTS):
        ids = np.nonzero(flat_e == e)[0][:C_REF]
        per_expert.append((ids, tok[ids], w_flat[ids]))
    return per_expert


def _pack_slots(per_expert):
    """Experts -> (core, slot); slot capacity = max token count in its column.

    Sorted-descending column assignment minimizes sum of column maxima;
    columns then ordered by capacity ascending (smallest xT load first).
    """
    loads = [len(t) for _, t, _ in per_expert]
    order = sorted(range(N_EXPERTS), key=lambda e: -loads[e])
    cols = [order[j * NCORE:(j + 1) * NCORE] for j in range(NSLOT)]
    cols.sort(key=lambda col: max(loads[e] for e in col))
    assign = np.empty((NCORE, NSLOT), np.int64)
    caps = []
    for j, col in enumerate(cols):
        for m in range(NCORE):
            assign[m, j] = col[m]
        caps.append(max(1, max(loads[e] for e in col)))
    return assign, tuple(caps)


def _prepare_core_inputs(x, per_expert, gup, down, assign, caps):
    x = np.ascontiguousarray(np.asarray(x, dtype=np.float32))
    gup = np.asarray(gup, dtype=np.float32)
    down = np.asarray(down, dtype=np.float32)
    xt_off = np.concatenate(
        [[0], np.cumsum([128 * KD * c for c in caps])]
    ).astype(np.int64)

    # per-expert weight layouts (each expert appears on exactly one core)
    gup_l, down_l = {}, {}
    for e in range(N_EXPERTS):
        halves = []
        for h in (0, 1):
            hm = gup[e, :, h::2]  # [DIM, INTER] gate or up, deinterleaved
            halves.append(
                hm.reshape(KD, 128, KI, 128).transpose(2, 1, 0, 3)
                .reshape(KI, 128, KD * 128)
            )
        gup_l[e] = np.stack(halves).astype(np.float16)  # [2, KI, 128, KD*128]
        down_l[e] = (
            down[e].reshape(KI, 128, NDT, 128).transpose(2, 1, 0, 3)
            .reshape(NDT, 128, KI * 128)
        ).astype(np.float16)

    in_maps = []
    for m in range(NCORE):
        xt_buf = np.zeros(xt_off[-1], np.float16)
        gup_buf = np.empty((NSLOT, 2, KI, 128, KD * 128), np.float16)
        down_buf = np.empty((NSLOT, NDT, 128, KI * 128), np.float16)
        for j in range(NSLOT):
            cap = caps[j]
            e = int(assign[m, j])
            _, toks, _ = per_expert[e]
            n = len(toks)
            xg = np.zeros((cap, DIM), np.float32)
            xg[:n] = x[toks]
            xt = xg.reshape(cap, KD, 128).transpose(2, 1, 0)  # [128, KD, cap]
            xt16 = xt.astype(np.float16)
            blk = 128 * SLABK * cap
            for s in range(NSLAB):
                xt_buf[xt_off[j] + s * blk: xt_off[j] + (s + 1) * blk] = (
                    np.ascontiguousarray(xt16[:, s * SLABK:(s + 1) * SLABK]).ravel()
                )
            gup_buf[j] = gup_l[e]
            down_buf[j] = down_l[e]
        in_maps.append({"xt": xt_buf, "gup": gup_buf, "down": down_buf})
    return in_maps


def _run(inputs: dict, trace: bool = False, tmpdir=None):
    from concourse.bass_utils import run_bass_kernel_spmd

    per_expert = _route(inputs["indices"], inputs["token_mask"], inputs["weights"])
    assign, caps = _pack_slots(per_expert)

    if caps not in _PROG_CACHE:
        _PROG_CACHE[caps] = _build_program(caps)
    nc, names = _PROG_CACHE[caps]

    core_maps = _prepare_core_inputs(
        inputs["x"], per_expert,
        inputs["gate_and_up_projs"], inputs["down_projs"], assign, caps,
    )
    in_maps = [{names[k]: v for k, v in mm.items()} for mm in core_maps]
    res = run_bass_kernel_spmd(
        nc, in_maps, list(range(NCORE)), trace=trace, tmpdir=tmpdir,
    )

    SC = sum(caps)
    soff = np.concatenate([[0], np.cumsum(caps)]).tolist()
    # y^T per core: [NDT, 128, SC] == [DIM, SC]; concat token columns
    ys = [np.asarray(res.results[m][names["y"]]).reshape(DIM, SC)
          for m in range(NCORE)]
    YT = np.concatenate(ys + [np.zeros((DIM, 1), ys[0].dtype)], axis=1)
    Yt = np.ascontiguousarray(YT.T, dtype=np.float32)  # [NCORE*SC + 1, DIM]

    T = N_TOKENS * TOPK
    pos = np.full(T, NCORE * SC, np.int64)  # default: zero column
    wts = np.zeros(T, np.float32)
    slot_of = {int(assign[m, j]): (m, j) for m in range(NCORE) for j in range(NSLOT)}
    for e in range(N_EXPERTS):
        ids, _, ws = per_expert[e]
        m, j = slot_of[e]
        pos[ids] = m * SC + soff[j] + np.arange(len(ids))
        wts[ids] = ws / ALPHA
    contrib = Yt[pos] * wts[:, None]
    out = contrib.reshape(N_TOKENS, TOPK, DIM).sum(axis=1, dtype=np.float32)
    return out.astype(np.float32), res


def kernel(**inputs) -> np.ndarray:
    out, _ = _run(inputs, trace=False)
    return out


# revision 10
# speedup vs baseline: 1.0287x; 1.0287x over previous
"""MoE grouped-experts kernel for Trainium2 (8 NeuronCores, expert-parallel).

v3 strategy
-----------
Expert-parallel, 32 experts on 8 cores x 4 slots, with:

- token-exact slot capacities: slot j's capacity = max over cores of the
  assigned expert's token count (no 128-row block quantization; padding
  drops from ~19% to ~2%). Slots sorted by capacity ascending so the
  first slot's xT load (the only non-overlapped DMA) is the smallest.
- fp16 matmul operands everywhere (same 1 col/cycle PE rate as fp32r,
  but half the HBM traffic and 2x faster FWL weight loads).
- GEMM1 (unchanged orientation): hT[m, t] = sum_k gup[k, m] * xT[k, t];
  stationary = gup tile, moving = token columns.
- activation: fg = Silu(alpha * min(gate, 7)) = alpha*glu;
  at = (clip(up) + 1) * fg  -> alpha * a_ref (probs NOT applied here).
- GEMM2 (transposed vs v1): yT[d, t] = sum_k down[k, d] * aT[k, t];
  stationary = down tile, moving = token columns. Output y^T [DIM, SC]
  keeps everything token-column-exact.
- routing probs (ws / alpha) applied on HOST during the combine
  (linear in y), removing the on-device per-token scale entirely.
"""

import math
from contextlib import ExitStack

import numpy as np

N_TOKENS, DIM = 4096, 2048
N_EXPERTS, TOPK, INTER = 32, 4, 1408
ALPHA, LIMIT, LIN_OFFSET = 1.702, 7.0, 1.0

NCORE = 8
NSLOT = N_EXPERTS // NCORE        # 4
KD = DIM // 128                   # 16 contraction tiles for GEMM1
KI = INTER // 128                 # 11 contraction tiles for GEMM2
NDT = DIM // 128                  # 16 output-dim tiles for GEMM2
SLABK = 8                         # k-tiles per xT slab (8.7KB DMA lines)
NSLAB = KD // SLABK               # 2
C_REF = 2 * ((N_TOKENS * TOPK + N_EXPERTS - 1) // N_EXPERTS)  # 1024

_PROG_CACHE: dict = {}


def _token_groups(cap: int):
    """Split cap into balanced moving-dim groups of <= 512 columns."""
    ng = max(1, math.ceil(cap / 512))
    base, rem = divmod(cap, ng)
    sizes = [base + (1 if i < rem else 0) for i in range(ng)]
    out, off = [], 0
    for s in sizes:
        out.append((off, s))
        off += s
    return out


def _build_program(caps: tuple):
    import concourse.bacc as bacc
    import concourse.mybir as mybir
    import concourse.tile as tile
    from concourse.alu_op_type import AluOpType

    F32 = mybir.dt.float32
    F16 = mybir.dt.float16
    capmax = max(caps)
    SC = sum(caps)
    soff = np.concatenate([[0], np.cumsum(caps)]).tolist()
    xt_off = np.concatenate(
        [[0], np.cumsum([128 * KD * c for c in caps])]
    ).tolist()

    nc = bacc.Bacc(None, target_bir_lowering=False, debug=False)
    with ExitStack() as ctx:
        tc = ctx.enter_context(tile.TileContext(nc))
        dram = ctx.enter_context(tc.tile_pool(name="dram", bufs=1, space="DRAM"))
        xt_d = dram.tile([xt_off[-1]], F16, kind="ExternalInput")
        gup_d = dram.tile([NSLOT, 2, KI, 128, KD * 128], F16, kind="ExternalInput")
        down_d = dram.tile([NSLOT, NDT, 128, KI * 128], F16, kind="ExternalInput")
        y_d = dram.tile([NDT, 128, SC], F16, kind="ExternalOutput")
        names = {
            "xt": xt_d.name, "gup": gup_d.name, "down": down_d.name, "y": y_d.name,
        }

        xt_pool = ctx.enter_context(tc.tile_pool(name="xt", bufs=2 * NSLAB))
        gup_pool = ctx.enter_context(tc.tile_pool(name="gup", bufs=6))
        down_pool = ctx.enter_context(tc.tile_pool(name="down", bufs=20))
        at_pool = ctx.enter_context(tc.tile_pool(name="at", bufs=2))
        fg_pool = ctx.enter_context(tc.tile_pool(name="fg", bufs=4))
        tmp_pool = ctx.enter_context(tc.tile_pool(name="tmp", bufs=4))
        yt_pool = ctx.enter_context(tc.tile_pool(name="yt", bufs=4))
        psg1 = ctx.enter_context(tc.tile_pool(name="psg1", bufs=4, space="PSUM"))
        psg2 = ctx.enter_context(tc.tile_pool(name="psg2", bufs=3, space="PSUM"))

        def emit_gemm1(j):
            cap = caps[j]
            groups = _token_groups(cap)

            xts, xt_dmas = [], []
            for s in range(NSLAB):
                t = xt_pool.tile([128, SLABK * capmax], F16, tag="xt")
                base = xt_off[j] + s * 128 * SLABK * cap

                def dma(t=t, base=base, cap=cap):
                    nc.sync.dma_start(
                        out=t[:, :SLABK * cap],
                        in_=xt_d[base: base + 128 * SLABK * cap].rearrange(
                            "(p c) -> p c", p=128
                        ),
                    )
                xt_dmas.append(dma)
                xts.append(t)
            # first slot: interleave the first gup load between the xT slabs
            # so the PE's first matmul isn't queued behind the whole xT image
            if j > 0:
                for d in xt_dmas:
                    d()
                xt_dmas = []
            else:
                xt_dmas[0]()
                xt_dmas = xt_dmas[1:]

            def xt_ap(k, g0, gw, cap=cap, xts=xts):
                t = xts[k // SLABK]
                kk = k % SLABK
                return t[:, kk * cap + g0: kk * cap + g0 + gw]

            at_sb = at_pool.tile([128, KI * capmax], F16, tag="at")

            for i in range(KI):
                pss = []
                for half in (0, 1):  # 0 = gate, 1 = up
                    gsb = gup_pool.tile([128, KD * 128], F16, tag="gup")
                    nc.sync.dma_start(out=gsb[:], in_=gup_d[j, half, i])
                    for d in xt_dmas:  # remaining first-slot slabs
                        d()
                    xt_dmas = []
                    ps_h = [
                        psg1.tile([128, 512], F32, tag="ps1",
                                  name=f"ps1_{j}_{i}_{half}_{gi}")
                        for gi in range(len(groups))
                    ]
                    for k in range(KD):
                        for gi, (g0, gw) in enumerate(groups):
                            nc.tensor.matmul(
                                ps_h[gi][:, :gw],
                                lhsT=gsb[:, k * 128:(k + 1) * 128],
                                rhs=xt_ap(k, g0, gw),
                                start=(k == 0), stop=(k == KD - 1),
                            )
                    pss.append(ps_h)
                for gi, (g0, gw) in enumerate(groups):
                    t0 = tmp_pool.tile([128, 512], F32, tag="t0")
                    nc.vector.tensor_scalar_min(t0[:, :gw], pss[0][gi][:, :gw], LIMIT)
                    fg = fg_pool.tile([128, 512], F32, tag="fg")
                    nc.scalar.activation(
                        fg[:, :gw], t0[:, :gw],
                        mybir.ActivationFunctionType.Silu, scale=ALPHA,
                    )
                    uc = tmp_pool.tile([128, 512], F32, tag="uc")
                    nc.vector.tensor_scalar(
                        uc[:, :gw], pss[1][gi][:, :gw], LIMIT, -LIMIT,
                        AluOpType.min, AluOpType.max,
                    )
                    # at = (clip(up)+1) * alpha*glu   (probs applied on host)
                    nc.vector.scalar_tensor_tensor(
                        at_sb[:, i * cap + g0: i * cap + g0 + gw],
                        uc[:, :gw], LIN_OFFSET, fg[:, :gw],
                        AluOpType.add, AluOpType.mult,
                    )
            return at_sb

        def emit_gemm2(j, at_sb):
            cap = caps[j]
            groups = _token_groups(cap)
            for dt in range(NDT):
                dsb = down_pool.tile([128, KI * 128], F16, tag="down")
                nc.sync.dma_start(out=dsb[:], in_=down_d[j, dt])
                for gi, (g0, gw) in enumerate(groups):
                    ps2 = psg2.tile([128, 512], F32, tag="ps2",
                                    name=f"ps2_{j}_{dt}_{gi}")
                    for k in range(KI):
                        nc.tensor.matmul(
                            ps2[:, :gw],
                            lhsT=dsb[:, k * 128:(k + 1) * 128],
                            rhs=at_sb[:, k * cap + g0: k * cap + g0 + gw],
                            start=(k == 0), stop=(k == KI - 1),
                        )
                    yt = yt_pool.tile([128, 512], F16, tag="yt")
                    nc.scalar.activation(
                        yt[:, :gw], ps2[:, :gw],
                        mybir.ActivationFunctionType.Copy,
                    )
                    nc.sync.dma_start(
                        out=y_d[dt, :, soff[j] + g0: soff[j] + g0 + gw],
                        in_=yt[:, :gw],
                    )

        # software pipeline: GEMM2 of slot j-1 is emitted after GEMM1 of
        # slot j, hiding the at->GEMM2 dependency tail and the slot
        # transition under the next slot's matmul stream
        prev_at = None
        for j in range(NSLOT + 1):
            cur_at = emit_gemm1(j) if j < NSLOT else None
            if j > 0:
                emit_gemm2(j - 1, prev_at)
            prev_at = cur_at
    nc.compile()
    return nc, names


def _route(indices, token_mask, weights):
    """Replicate the reference's permute/capacity semantics on host."""
    idx = np.asarray(indices).astype(np.int64)
    mask = np.asarray(token_mask).astype(bool)
    w = np.asarray(weights).astype(np.float32)
    flat_e = np.where(mask[:, None], idx, -1).ravel()
    w_flat = np.where(flat_e >= 0, w.ravel(), 0.0).astype(np.float32)

    per_expert = []  # (flat_ids, token_ids, weights), flat order, capped at C_REF
    tok = np.repeat(np.arange(N_TOKENS, dtype=np.int64), TOPK)
    for e in range(N_EXPERTS):
        ids = np.nonzero(flat_e == e)[0][:C_REF]
        per_expert.append((ids, tok[ids], w_flat[ids]))
    return per_expert


def _pack_slots(per_expert):
    """Experts -> (core, slot); slot capacity = max token count in its column.

    Sorted-descending column assignment minimizes sum of column maxima;
    columns then ordered by capacity ascending (smallest xT load first).
    """
    loads = [len(t) for _, t, _ in per_expert]
    order = sorted(range(N_EXPERTS), key=lambda e: -loads[e])
    cols = [order[j * NCORE:(j + 1) * NCORE] for j in range(NSLOT)]
    cols.sort(key=lambda col: max(loads[e] for e in col))
    assign = np.empty((NCORE, NSLOT), np.int64)
    caps = []
    for j, col in enumerate(cols):
        for m in range(NCORE):
            assign[m, j] = col[m]
        caps.append(max(1, max(loads[e] for e in col)))
    return assign, tuple(caps)


def _prepare_core_inputs(x, per_expert, gup, down, assign, caps):
    x = np.ascontiguousarray(np.asarray(x, dtype=np.float32))
    gup = np.asarray(gup, dtype=np.float32)
    down = np.asarray(down, dtype=np.float32)
    xt_off = np.concatenate(
        [[0], np.cumsum([128 * KD * c for c in caps])]
    ).astype(np.int64)

    # per-expert weight layouts (each expert appears on exactly one core)
    gup_l, down_l = {}, {}
    for e in range(N_EXPERTS):
        halves = []
        for h in (0, 1):
            hm = gup[e, :, h::2]  # [DIM, INTER] gate or up, deinterleaved
            halves.append(
                hm.reshape(KD, 128, KI, 128).transpose(2, 1, 0, 3)
                .reshape(KI, 128, KD * 128)
            )
        gup_l[e] = np.stack(halves).astype(np.float16)  # [2, KI, 128, KD*128]
        down_l[e] = (
            down[e].reshape(KI, 128, NDT, 128).transpose(2, 1, 0, 3)
            .reshape(NDT, 128, KI * 128)
        ).astype(np.float16)

    in_maps = []
    for m in range(NCORE):
        xt_buf = np.zeros(xt_off[-1], np.float16)
        gup_buf = np.empty((NSLOT, 2, KI, 128, KD * 128), np.float16)
        down_buf = np.empty((NSLOT, NDT, 128, KI * 128), np.float16)
        for j in range(NSLOT):
            cap = caps[j]
            e = int(assign[m, j])
            _, toks, _ = per_expert[e]
            n = len(toks)
            xg = np.zeros((cap, DIM), np.float32)
            xg[:n] = x[toks]
            xt = xg.reshape(cap, KD, 128).transpose(2, 1, 0)  # [128, KD, cap]
            xt16 = xt.astype(np.float16)
            blk = 128 * SLABK * cap
            for s in range(NSLAB):
                xt_buf[xt_off[j] + s * blk: xt_off[j] + (s + 1) * blk] = (
                    np.ascontiguousarray(xt16[:, s * SLABK:(s + 1) * SLABK]).ravel()
                )
            gup_buf[j] = gup_l[e]
            down_buf[j] = down_l[e]
        in_maps.append({"xt": xt_buf, "gup": gup_buf, "down": down_buf})
    return in_maps


def _run(inputs: dict, trace: bool = False, tmpdir=None):
    from concourse.bass_utils import run_bass_kernel_spmd

    per_expert = _route(inputs["indices"], inputs["token_mask"], inputs["weights"])
    assign, caps = _pack_slots(per_expert)

    if caps not in _PROG_CACHE:
        _PROG_CACHE[caps] = _build_program(caps)
    nc, names = _PROG_CACHE[caps]

    core_maps = _prepare_core_inputs(
        inputs["x"], per_expert,
        inputs["gate_and_up_projs"], inputs["down_projs"], assign, caps,
    )
    in_maps = [{names[k]: v for k, v in mm.items()} for mm in core_maps]
    res = run_bass_kernel_spmd(
        nc, in_maps, list(range(NCORE)), trace=trace, tmpdir=tmpdir,
    )

    SC = sum(caps)
    soff = np.concatenate([[0], np.cumsum(caps)]).tolist()
    # y^T per core: [NDT, 128, SC] == [DIM, SC]; concat token columns
    ys = [np.asarray(res.results[m][names["y"]]).reshape(DIM, SC)
          for m in range(NCORE)]
    YT = np.concatenate(ys + [np.zeros((DIM, 1), ys[0].dtype)], axis=1)
    Yt = np.ascontiguousarray(YT.T, dtype=np.float32)  # [NCORE*SC + 1, DIM]

    T = N_TOKENS * TOPK
    pos = np.full(T, NCORE * SC, np.int64)  # default: zero column
    wts = np.zeros(T, np.float32)
    slot_of = {int(assign[m, j]): (m, j) for m in range(NCORE) for j in range(NSLOT)}
    for e in range(N_EXPERTS):
        ids, _, ws = per_expert[e]
        m, j = slot_of[e]
        pos[ids] = m * SC + soff[j] + np.arange(len(ids))
        wts[ids] = ws / ALPHA
    contrib = Yt[pos] * wts[:, None]
    out = contrib.reshape(N_TOKENS, TOPK, DIM).sum(axis=1, dtype=np.float32)
    return out.astype(np.float32), res


def kernel(**inputs) -> np.ndarray:
    out, _ = _run(inputs, trace=False)
    return out
